# revision 1
# baseline (speedup 1.0000x reference)
"""Multi-Head Latent Attention (DeepSeek-style MLA) on 8 TRN2 NeuronCores.

Sharding: core c handles batch b = c//2 and query rows [ (c%2)*S/2, (c%2+1)*S/2 ).
Each core computes the full KV-side projections for its batch (duplicated between
the two cores sharing a batch) and the Q-side projections / attention / output
projection for its query half. No collectives; the host gathers the 8 output
shards.

Layout strategy: activations are kept feature-major ("transposed", [feature, seq])
so every matmul's contraction dim lands on SBUF partitions. Attention output is
produced directly as attT[h*128+d, q] (v as stationary operand, expT as moving),
which is exactly the lhsT layout the output projection needs - no PE transposes
anywhere. Softmax skips the max-subtraction (scores here are O(1); exp is safe)
and the denominator comes from an all-ones stationary matmul over expT.

RoPE is folded into companion weight matrices host-side:
  rope(x)[2i]   = x[2i] cos_i - x[2i+1] sin_i
  rope(x)[2i+1] = x[2i+1] cos_i + x[2i] sin_i
so with xr = x @ Wr where Wr[:,2i] = -W[:,2i+1], Wr[:,2i+1] = W[:,2i]:
  rope(x @ W) = (x @ W) * cosP + (x @ Wr) * sinP   (pure elementwise).

Matmuls run as float32r (fp32 storage, full PE rate at N>=256), except the
64-row rotary score matmuls which run in bf16 (fp32r pays 2 cycles/row below
K=96; bf16 keeps full rate and only touches the small rotary contribution).
"""

import sys
import numpy as np

sys.path.insert(0, "/opt/trn_rl_repo")

from contextlib import ExitStack  # noqa: E402

import concourse.bass as bass  # noqa: E402
import concourse.mybir as mybir  # noqa: E402
import concourse.tile as tile  # noqa: E402

F32 = mybir.dt.float32
FR = mybir.dt.float32r
BF = mybir.dt.bfloat16
AF = mybir.ActivationFunctionType
ALU = mybir.AluOpType

# Max sync-waits walrus CoreV3 codegen accepts on one instruction. The stock
# TileContext tail-drain attaches one wait per outstanding semaphore to a
# single Drain, which this walrus build rejects ("Too many sync wait
# commands"); split across several drains instead.
_MAX_WAITS_PER_INST = 1


def _split_excess_waits_json(bir_json):
    """Walrus CoreV3 codegen rejects instructions carrying more than one
    sync-wait. Tile freely attaches several. Rewrite the BIR: keep one wait on
    the instruction, move the rest onto NoOps inserted just before it on the
    same engine (a same-engine wait that fires earlier is strictly safe).
    Updates are left untouched - they must fire at instruction completion."""
    import orjson

    bir = orjson.loads(bir_json)
    n = 0
    for fn in bir.get("functions", []):
        for bb in fn.get("blocks", []):
            out = []
            for inst in bb.get("instructions", []):
                si = inst.get("sync_info")
                waits = (si or {}).get("on_wait") or []
                if len(waits) > _MAX_WAITS_PER_INST:
                    keep = waits[-_MAX_WAITS_PER_INST:]
                    for w in waits[:-_MAX_WAITS_PER_INST]:
                        out.append({
                            "name": f"I-WS{n}",
                            "opcode": "NoOp",
                            "engine": inst["engine"],
                            "ins": [],
                            "outs": [],
                            "sync_info": {"on_update": [], "on_wait": [w]},
                        })
                        n += 1
                    si["on_wait"] = keep
                out.append(inst)
            bb["instructions"] = out
    return orjson.dumps(bir)


_COMPILE_HOOKED = False


def _install_wait_split_hook():
    """Wrap compile_bir_kernel (both the bass_utils global and the name
    bass2jax imported) so every BIR headed to walrus gets the wait split."""
    global _COMPILE_HOOKED
    if _COMPILE_HOOKED:
        return
    from concourse import bass2jax, bass_utils

    orig = bass_utils.compile_bir_kernel

    def hooked(bir_json, tmpdir, neff_name="file.neff"):
        return orig(_split_excess_waits_json(bir_json), tmpdir, neff_name=neff_name)

    bass_utils.compile_bir_kernel = hooked
    bass2jax.compile_bir_kernel = hooked
    _COMPILE_HOOKED = True


class SplitDrainTileContext(tile.TileContext):
    def _drain_and_barrier(self, tick_clock, wait_clock):
        from concourse.tile_scheduler import N_PROCS
        from concourse.vector_clock import ScopedClock, VectorClock

        g = tick_clock.global_clock
        vals = [g[p] for p in range(N_PROCS)]
        nz = [p for p in range(N_PROCS) if vals[p] > 0]
        groups = [nz[i:i + _MAX_WAITS_PER_INST]
                  for i in range(0, len(nz), _MAX_WAITS_PER_INST)] or [[]]
        for grp in groups:
            sub = VectorClock([vals[p] if p in grp else 0 for p in range(N_PROCS)])
            drain_inst = self.nc.sync.drain()
            wait_clock.add_sem_waits(drain_inst.ins, ScopedClock({None: sub}))

        self.nc.all_engine_barrier()
        assert self.sems is not None
        popped = self.nc._tile_sem_poison_stack.pop()
        assert popped is self._sem_poison
        self.nc.clear_and_free_semaphores(list(self.sems.allocated().values()))
        self.nc.all_engine_barrier()


# ----------------------------------------------------------------------------
# Config
# ----------------------------------------------------------------------------

class Cfg:
    def __init__(self, E=2048, DM=2048, H=16, DC=512, DC1=1536, S=2048, Q=1024,
                 QT=512, mm_dt=mybir.dt.float32r):
        self.E, self.DM, self.H, self.DC, self.DC1 = E, DM, H, DC, DC1
        self.S, self.Q, self.QT = S, Q, QT
        self.DR = 64          # rotary dim (fixed by the problem)
        self.DH = 128         # nope head dim (fixed: DM // H)
        self.mm_dt = mm_dt
        assert DM == H * self.DH and H % 2 == 0
        assert E % 128 == 0 and DC % 128 == 0 and DC1 % 128 == 0
        assert S % 128 == 0
        assert Q % QT == 0 and Q % 128 == 0 and QT <= 512
        self.EC = E // 128        # embed chunks
        self.CC = DC // 128       # c_kv chunks
        self.C1C = DC1 // 128     # c_q chunks
        self.KC = S // 128        # key chunks (128-wide)
        self.ST = min(512, S)     # seq tile for phase 1
        self.STN = S // self.ST
        self.NT = min(512, S)     # kT free tile
        self.NTN = S // self.NT
        self.QTN = Q // QT
        self.MT = min(512, DM)    # out-proj free tile
        self.MTN = DM // self.MT
        self.QON = Q // 128       # out-proj q tiles


FULL = Cfg()


# ----------------------------------------------------------------------------
# Program builder (single-core SPMD program)
# ----------------------------------------------------------------------------

def build_program(cfg: Cfg, has_buv=True, has_bo=True):
    c = cfg
    FR = BF if getattr(cfg, "bf16", False) else mybir.dt.float32r  # noqa: F841
    nc = bass.Bass()
    r = lambda ap: ap  # noqa: E731  (tiles already float32r)

    # -- DRAM parameters -----------------------------------------------------
    xt = nc.dram_tensor("xt", [c.E, c.S], FR, kind="ExternalInput")
    xtq = nc.dram_tensor("xtq", [c.E, c.Q], FR, kind="ExternalInput")
    cosq = nc.dram_tensor("cosq", [128, c.Q], F32, kind="ExternalInput")
    sinq = nc.dram_tensor("sinq", [128, c.Q], F32, kind="ExternalInput")
    cosk = nc.dram_tensor("cosk", [64, c.S], F32, kind="ExternalInput")
    sink = nc.dram_tensor("sink", [64, c.S], F32, kind="ExternalInput")
    wdq = nc.dram_tensor("wdq", [c.E, c.DC1], FR, kind="ExternalInput")
    bdq = nc.dram_tensor("bdq", [c.DC1], F32, kind="ExternalInput")
    wdkv = nc.dram_tensor("wdkv", [c.E, c.DC], FR, kind="ExternalInput")
    bdkv = nc.dram_tensor("bdkv", [c.DC], F32, kind="ExternalInput")
    wuq = nc.dram_tensor("wuq", [c.DC1, c.DM], FR, kind="ExternalInput")
    buq = nc.dram_tensor("buq", [c.DM], F32, kind="ExternalInput")
    wrq = nc.dram_tensor("wrq", [c.DC1, c.H * c.DR], FR, kind="ExternalInput")
    brq = nc.dram_tensor("brq", [c.H * c.DR], F32, kind="ExternalInput")
    wrqr = nc.dram_tensor("wrqr", [c.DC1, c.H * c.DR], FR, kind="ExternalInput")
    brqr = nc.dram_tensor("brqr", [c.H * c.DR], F32, kind="ExternalInput")
    wrk = nc.dram_tensor("wrk", [c.E, 2 * c.DR], FR, kind="ExternalInput")
    brk = nc.dram_tensor("brk", [2 * c.DR], F32, kind="ExternalInput")
    wuk = nc.dram_tensor("wuk", [c.DC, c.DM], FR, kind="ExternalInput")
    buk = nc.dram_tensor("buk", [c.DM], F32, kind="ExternalInput")
    wuv = nc.dram_tensor("wuv", [c.DC, c.DM], FR, kind="ExternalInput")
    buv = nc.dram_tensor("buv", [c.DM], FR, kind="ExternalInput")
    wo = nc.dram_tensor("wo", [c.DM, c.DM], FR, kind="ExternalInput")
    bo = nc.dram_tensor("bo", [c.DM], FR, kind="ExternalInput")
    ones_d = nc.dram_tensor("ones_in", [128, 128], FR, kind="ExternalInput")
    out = nc.dram_tensor("out", [c.Q, c.DM], F32, kind="ExternalOutput")
    attT = nc.dram_tensor("attT_scratch", [c.DM, c.Q], FR)
    qt_d = nc.dram_tensor("qT_scratch", [c.DM, c.Q], FR)
    qrt_d = nc.dram_tensor("qrT_scratch", [c.DM // 2, c.Q], BF)

    with SplitDrainTileContext(nc) as tc, ExitStack() as ctx:
        # -- persistent pools ------------------------------------------------
        consts = ctx.enter_context(tc.tile_pool(name="consts", bufs=1))
        res = ctx.enter_context(tc.tile_pool(name="res", bufs=1))

        ckvT = res.tile([128, c.CC, c.S], FR, tag="ckvT")     # c_kv^T
        krT = res.tile([128, c.S], BF, tag="krT")             # roped k_rot^T, dup rows

        ones128 = consts.tile([128, 128], FR, tag="ones128")
        nc.sync.dma_start(out=ones128, in_=ones_d[:, :])
        ones1 = ones128[0:1, :]

        def load_pcol(name, vec, n):
            # [n*128] dram vector -> [128, n] sbuf (per-partition scalars)
            t = consts.tile([128, n], F32, tag=name)
            nc.sync.dma_start(out=t, in_=vec.rearrange("(c p) -> p c", p=128))
            return t

        bdq_sb = load_pcol("bdq", bdq, c.C1C)
        bdkv_sb = load_pcol("bdkv", bdkv, c.CC)
        buq_sb = load_pcol("buq", buq, c.H)
        brq_sb = load_pcol("brq", brq, c.H // 2)
        brqr_sb = load_pcol("brqr", brqr, c.H // 2)
        # packed k-rope bias: col 0 = brk[0:64], col 1 = companion brk[64:128],
        # both based at partition 0 (DVE ops need same start partition)
        brk_sb = consts.tile([64, 2], F32, tag="brk")
        nc.sync.dma_start(out=brk_sb, in_=brk.rearrange("(c p) -> p c", p=64))
        buk_sb = load_pcol("buk", buk, c.H)
        buv_sb = consts.tile([1, c.DM], FR, tag="buv")
        nc.sync.dma_start(out=buv_sb, in_=buv[:].unsqueeze(0))
        bo_sb = consts.tile([1, c.DM], FR, tag="bo")
        nc.sync.dma_start(out=bo_sb, in_=bo[:].unsqueeze(0))

        # PSUM pools (8 banks total: 2+2+2+2)
        psA = ctx.enter_context(tc.tile_pool(name="psA", bufs=2, space="PSUM"))
        psS = ctx.enter_context(tc.tile_pool(name="psS", bufs=3, space="PSUM"))
        psG = ctx.enter_context(tc.tile_pool(name="psG", bufs=2, space="PSUM"))
        psZ = ctx.enter_context(tc.tile_pool(name="psZ", bufs=1, space="PSUM"))

        # ==================================================================
        # Phase 1a: c_kv^T and roped k_rot^T over the full sequence
        # ==================================================================
        with tc.tile_pool(name="p1ax", bufs=c.EC + 4) as p1ax, \
             tc.tile_pool(name="p1aw", bufs=c.EC) as p1aw, \
             tc.tile_pool(name="p1am", bufs=1) as p1am, \
             tc.tile_pool(name="p1at", bufs=4) as p1at:

            cosk_sb = p1am.tile([64, c.S], F32, tag="cosk")
            sink_sb = p1am.tile([64, c.S], F32, tag="sink")
            nc.sync.dma_start(out=cosk_sb, in_=cosk[:, :])
            nc.sync.dma_start(out=sink_sb, in_=sink[:, :])

            wdkv_t, wrk_t = [], []
            for e in range(c.EC):
                wt = p1aw.tile([128, c.DC], FR, tag="wdkv")
                nc.sync.dma_start(out=wt, in_=wdkv[e * 128:(e + 1) * 128, :])
                wdkv_t.append(wt)
                rt = p1aw.tile([128, 2 * c.DR], FR, tag="wrk")
                nc.sync.dma_start(out=rt, in_=wrk[e * 128:(e + 1) * 128, :])
                wrk_t.append(rt)

            for st in range(c.STN):
                ssl = bass.ts(st, c.ST)
                xts = []
                for e in range(c.EC):
                    t = p1ax.tile([128, c.ST], FR, tag="xt")
                    nc.sync.dma_start(out=t, in_=xt[e * 128:(e + 1) * 128, ssl])
                    xts.append(t)
                for ct in range(c.CC):
                    ps = psA.tile([128, c.ST], F32, tag="ps")
                    for e in range(c.EC):
                        nc.tensor.matmul(ps, r(wdkv_t[e][:, ct * 128:(ct + 1) * 128]),
                                         r(xts[e]), start=(e == 0), stop=(e == c.EC - 1))
                    nc.vector.tensor_scalar_add(ckvT[:, ct, ssl], ps,
                                                bdkv_sb[:, ct:ct + 1])
                # k_rot: A rows and Ar rows in separate psums (partition-aligned)
                psa = psA.tile([64, c.ST], F32, tag="ps")
                for e in range(c.EC):
                    nc.tensor.matmul(psa, r(wrk_t[e][:, 0:c.DR]), r(xts[e]),
                                     start=(e == 0), stop=(e == c.EC - 1))
                psar = psA.tile([64, c.ST], F32, tag="ps")
                for e in range(c.EC):
                    nc.tensor.matmul(psar, r(wrk_t[e][:, c.DR:2 * c.DR]), r(xts[e]),
                                     start=(e == 0), stop=(e == c.EC - 1))
                tmp = p1at.tile([64, c.ST], F32, tag="ktmp")
                nc.vector.scalar_tensor_tensor(tmp, psa, brk_sb[:, 0:1],
                                               cosk_sb[:, ssl], ALU.add, ALU.mult)
                nc.vector.scalar_tensor_tensor(krT[0:64, ssl], psar,
                                               brk_sb[:, 1:2],
                                               sink_sb[:, ssl], ALU.add, ALU.mult)
                nc.vector.tensor_add(krT[0:64, ssl], krT[0:64, ssl], tmp)
            # duplicate kr rows so odd heads can matmul at base_partition 64
            nc.sync.dma_start(out=krT[64:128, :], in_=krT[0:64, :])

        # ==================================================================
        # Phase 1b/1c: c_q^T, then q^T (scaled) and roped q_rot^T -> DRAM
        # ==================================================================
        with tc.tile_pool(name="pcq", bufs=1) as pcq:
            cqT = pcq.tile([128, c.C1C, c.Q], FR, tag="cqT")

            with tc.tile_pool(name="p1bx", bufs=c.QTN * c.EC + 2) as p1bx, \
                 tc.tile_pool(name="p1bw", bufs=3) as p1bw:
                # all query-tile activations resident so wdq streams ONCE
                xqs = {}
                for qt in range(c.QTN):
                    qsl = bass.ts(qt, c.QT)
                    for e in range(c.EC):
                        t = p1bx.tile([128, c.QT], FR, tag="xq")
                        nc.sync.dma_start(out=t, in_=xtq[e * 128:(e + 1) * 128, qsl])
                        xqs[qt, e] = t
                for ct in range(c.C1C):
                    wdq_ct = p1bw.tile([128, c.EC, 128], FR, tag="wdq")
                    nc.sync.dma_start(
                        out=wdq_ct,
                        in_=wdq.rearrange("(e p) m -> p e m", p=128)[:, :, ct * 128:(ct + 1) * 128])
                    for qt in range(c.QTN):
                        qsl = bass.ts(qt, c.QT)
                        ps = psA.tile([128, c.QT], F32, tag="ps")
                        for e in range(c.EC):
                            nc.tensor.matmul(ps, r(wdq_ct[:, e, :]), r(xqs[qt, e]),
                                             start=(e == 0), stop=(e == c.EC - 1))
                        nc.vector.tensor_scalar_add(cqT[:, ct, qsl], ps,
                                                    bdq_sb[:, ct:ct + 1])

            with tc.tile_pool(name="p1cw", bufs=3) as p1cw, \
                 tc.tile_pool(name="p1cm", bufs=1) as p1cm, \
                 tc.tile_pool(name="p1cs", bufs=3) as p1cs, \
                 tc.tile_pool(name="p1ct", bufs=4) as p1ct:

                cosq_sb = p1cm.tile([128, c.Q], F32, tag="cosq")
                sinq_sb = p1cm.tile([128, c.Q], F32, tag="sinq")
                nc.sync.dma_start(out=cosq_sb, in_=cosq[:, :])
                nc.sync.dma_start(out=sinq_sb, in_=sinq[:, :])

                for h in range(c.H):
                    wuq_h = p1cw.tile([128, c.C1C, 128], FR, tag="wuq")
                    nc.sync.dma_start(
                        out=wuq_h,
                        in_=wuq.rearrange("(cc p) m -> p cc m", p=128)[:, :, h * 128:(h + 1) * 128])
                    for qt in range(c.QTN):
                        qsl = bass.ts(qt, c.QT)
                        ps = psA.tile([128, c.QT], F32, tag="ps")
                        for ct in range(c.C1C):
                            nc.tensor.matmul(ps, r(wuq_h[:, ct, :]), r(cqT[:, ct, qsl]),
                                             start=(ct == 0), stop=(ct == c.C1C - 1))
                        ev = p1cs.tile([128, c.QT], FR, tag="qev")
                        nc.vector.tensor_scalar_add(ev, ps, buq_sb[:, h:h + 1])
                        nc.sync.dma_start(out=qt_d[h * 128:(h + 1) * 128, qsl], in_=ev)
                for hp in range(c.H // 2):
                    wrq_hp = p1cw.tile([128, c.C1C, 128], FR, tag="wrq")
                    nc.sync.dma_start(
                        out=wrq_hp,
                        in_=wrq.rearrange("(cc p) m -> p cc m", p=128)[:, :, hp * 128:(hp + 1) * 128])
                    wrqr_hp = p1cw.tile([128, c.C1C, 128], FR, tag="wrqr")
                    nc.sync.dma_start(
                        out=wrqr_hp,
                        in_=wrqr.rearrange("(cc p) m -> p cc m", p=128)[:, :, hp * 128:(hp + 1) * 128])
                    for qt in range(c.QTN):
                        qsl = bass.ts(qt, c.QT)
                        psa = psA.tile([128, c.QT], F32, tag="ps")
                        for ct in range(c.C1C):
                            nc.tensor.matmul(psa, r(wrq_hp[:, ct, :]), r(cqT[:, ct, qsl]),
                                             start=(ct == 0), stop=(ct == c.C1C - 1))
                        psar = psA.tile([128, c.QT], F32, tag="ps")
                        for ct in range(c.C1C):
                            nc.tensor.matmul(psar, r(wrqr_hp[:, ct, :]), r(cqT[:, ct, qsl]),
                                             start=(ct == 0), stop=(ct == c.C1C - 1))
                        tmp = p1ct.tile([128, c.QT], F32, tag="qtmp")
                        nc.vector.scalar_tensor_tensor(tmp, psa, brq_sb[:, hp:hp + 1],
                                                       cosq_sb[:, qsl], ALU.add, ALU.mult)
                        ev = p1cs.tile([128, c.QT], BF, tag="qrev")
                        nc.vector.scalar_tensor_tensor(ev, psar,
                                                       brqr_sb[:, hp:hp + 1],
                                                       sinq_sb[:, qsl], ALU.add, ALU.mult)
                        nc.vector.tensor_add(ev, ev, tmp)
                        nc.sync.dma_start(out=qrt_d[hp * 128:(hp + 1) * 128, qsl], in_=ev)

        # out-proj pools open early: mt=0's wo tiles prefetch during the
        # attention phase (DMA queues are ~90% idle there), so phase 3
        # starts without a weight-load stall.
        ow = ctx.enter_context(tc.tile_pool(name="ow", bufs=c.H + 6))
        oo = ctx.enter_context(tc.tile_pool(name="oo", bufs=3))
        oa = ctx.enter_context(tc.tile_pool(name="oa", bufs=2))
        wo_pre = []
        for hc in range(c.H):
            t = ow.tile([128, c.MT], FR, tag="wo")
            nc.sync.dma_start(out=t, in_=wo[hc * 128:(hc + 1) * 128, 0:c.MT])
            wo_pre.append(t)

        # ==================================================================
        # Phase 2: per-head attention
        # ==================================================================
        with tc.tile_pool(name="hw", bufs=3) as hw, \
             tc.tile_pool(name="hk", bufs=2) as hk, \
             tc.tile_pool(name="hq", bufs=2) as hq, \
             tc.tile_pool(name="hv", bufs=3) as hv, \
             tc.tile_pool(name="he", bufs=4) as he, \
             tc.tile_pool(name="hr", bufs=2) as hr:

            v_tiles = {}
            qr_tile = {}
            for h in range(c.H):
                hp, par = h // 2, (h % 2) * 64
                if h % 2 == 0:
                    # v for the head pair: [s, 2*128], 128 s-rows at a time
                    wuv_hp = hw.tile([128, c.CC, 256], FR, tag="wuv")
                    nc.sync.dma_start(
                        out=wuv_hp,
                        in_=wuv.rearrange("(cc p) m -> p cc m", p=128)[:, :, hp * 256:(hp + 1) * 256])
                    v0 = hv.tile([128, c.KC, 128], FR, tag="vh")
                    v1 = hv.tile([128, c.KC, 128], FR, tag="vh")
                    for st in range(c.KC):
                        ps = psA.tile([128, 256], F32, tag="ps")
                        for cc in range(c.CC):
                            nc.tensor.matmul(ps, r(ckvT[:, cc, st * 128:(st + 1) * 128]),
                                             r(wuv_hp[:, cc, :]),
                                             start=(cc == 0),
                                             stop=(not has_buv and cc == c.CC - 1))
                        if has_buv:
                            nc.tensor.matmul(ps, r(ones1),
                                             r(buv_sb[:, hp * 256:(hp + 1) * 256]),
                                             start=False, stop=True)
                        nc.vector.tensor_copy(v0[:, st, :], ps[:, 0:128])
                        nc.vector.tensor_copy(v1[:, st, :], ps[:, 128:256])
                    v_tiles[h], v_tiles[h + 1] = v0, v1
                    qr = hq.tile([128, c.Q], BF, tag="qr")
                    nc.sync.dma_start(out=qr, in_=qrt_d[hp * 128:(hp + 1) * 128, :])
                    qr_tile[h] = qr_tile[h + 1] = qr

                # kT for this head: [128 d, S]
                wuk_h = hw.tile([128, c.CC, 128], FR, tag="wuk")
                nc.sync.dma_start(
                    out=wuk_h,
                    in_=wuk.rearrange("(cc p) m -> p cc m", p=128)[:, :, h * 128:(h + 1) * 128])
                kT = hk.tile([128, c.S], FR, tag="kT")
                for nt in range(c.NTN):
                    nsl = bass.ts(nt, c.NT)
                    ps = psA.tile([128, c.NT], F32, tag="ps")
                    for cc in range(c.CC):
                        nc.tensor.matmul(ps, r(wuk_h[:, cc, :]), r(ckvT[:, cc, nsl]),
                                         start=(cc == 0), stop=(cc == c.CC - 1))
                    nc.vector.tensor_scalar_add(kT[:, nsl], ps, buk_sb[:, h:h + 1])

                qTh = hq.tile([128, c.Q], FR, tag="qTh")
                nc.sync.dma_start(out=qTh, in_=qt_d[h * 128:(h + 1) * 128, :])

                vh, qrh = v_tiles[h], qr_tile[h]
                for qt in range(c.QTN):
                    qsl = bass.ts(qt, c.QT)
                    gps = psG.tile([128, c.QT], F32, tag="g")
                    zps = psZ.tile([128, c.QT], F32, tag="z")
                    for kc in range(c.KC):
                        ksl = bass.ts(kc, 128)
                        sps = psS.tile([128, c.QT], F32, tag="s")
                        nc.tensor.matmul(sps, r(kT[:, ksl]), r(qTh[:, qsl]),
                                         start=True, stop=False)
                        nc.tensor.matmul(sps, r(krT[par:par + 64, ksl]),
                                         r(qrh[par:par + 64, qsl]),
                                         start=False, stop=True)
                        et = he.tile([128, c.QT], FR, tag="e")
                        nc.scalar.activation(et, sps, AF.Exp)
                        nc.tensor.matmul(gps, r(vh[:, kc, :]), r(et),
                                         start=(kc == 0), stop=(kc == c.KC - 1))
                        nc.tensor.matmul(zps, r(ones128), r(et),
                                         start=(kc == 0), stop=(kc == c.KC - 1))
                    recip = hr.tile([128, c.QT], F32, tag="recip")
                    nc.vector.reciprocal(recip, zps)
                    asb = hr.tile([128, c.QT], FR, tag="attsb")
                    nc.vector.tensor_mul(asb, gps, recip)
                    nc.sync.dma_start(out=attT[h * 128:(h + 1) * 128, qsl], in_=asb)

        # ==================================================================
        # Phase 3: output projection  out[q, m] = attT.T @ wo + bo
        # ==================================================================
        for mt in range(c.MTN):
            msl = bass.ts(mt, c.MT)
            if mt == 0:
                wo_t = wo_pre
            else:
                wo_t = []
                for hc in range(c.H):
                    t = ow.tile([128, c.MT], FR, tag="wo")
                    nc.sync.dma_start(out=t, in_=wo[hc * 128:(hc + 1) * 128, msl])
                    wo_t.append(t)
            for qo in range(c.QON):
                aq = oa.tile([128, c.H, 128], FR, tag="attq")
                nc.sync.dma_start(
                    out=aq,
                    in_=attT.rearrange("(hc p) q -> p hc q", p=128)[:, :, qo * 128:(qo + 1) * 128])
                ps = psA.tile([128, c.MT], F32, tag="ps")
                for hc in range(c.H):
                    nc.tensor.matmul(ps, r(aq[:, hc, :]), r(wo_t[hc]),
                                     start=(hc == 0),
                                     stop=(not has_bo and hc == c.H - 1))
                if has_bo:
                    nc.tensor.matmul(ps, r(ones1), r(bo_sb[:, msl]),
                                     start=False, stop=True)
                osb = oo.tile([128, c.MT], F32, tag="osb")
                nc.vector.tensor_copy(osb, ps)
                nc.sync.dma_start(out=out[qo * 128:(qo + 1) * 128, msl], in_=osb)

    return nc


# ----------------------------------------------------------------------------
# Host side: input prep, sharding, gather
# ----------------------------------------------------------------------------

def _rope_tables(seq_len, dim, theta=10000.0):
    inv_freq = 1.0 / (theta ** (np.arange(0, dim, 2, dtype=np.float32) / dim))
    t = np.arange(seq_len, dtype=np.float32)
    ang = t[:, None] * inv_freq[None, :]  # [S, dim/2]
    return np.cos(ang).astype(np.float32), np.sin(ang).astype(np.float32)


def _rot_companion_cols(w):
    """wr[..., 2i] = -w[..., 2i+1]; wr[..., 2i+1] = w[..., 2i]."""
    wr = np.empty_like(w)
    wr[..., 0::2] = -w[..., 1::2]
    wr[..., 1::2] = w[..., 0::2]
    return wr


def host_inputs(cfg, sequence, W_dkv, b_dkv, W_dq, b_dq, W_uq, b_uq, W_uk, b_uk,
                W_uv, b_uv, W_rq, b_rq, W_rk, b_rk, W_o, b_o):
    """Build the per-core input maps for the SPMD program."""
    c = cfg
    f = lambda a: np.ascontiguousarray(np.asarray(a, dtype=np.float32))  # noqa: E731
    sequence = f(sequence)
    B = sequence.shape[0]
    scaler = np.float32(1.0 / np.sqrt(c.DH + c.DR))

    cos, sin = _rope_tables(c.S, c.DR)  # [S, 32]
    # rows 2i and 2i+1 both carry table column i
    cosk = np.repeat(cos.T, 2, axis=0)  # [64, S]
    sink = np.repeat(sin.T, 2, axis=0)

    shared = dict(
        wdq=f(W_dq), bdq=f(b_dq),
        wdkv=f(W_dkv), bdkv=f(b_dkv),
        wuq=f(W_uq) * scaler, buq=f(b_uq) * scaler,
        wrq=f(W_rq) * scaler, brq=f(b_rq) * scaler,
        wrqr=_rot_companion_cols(f(W_rq) * scaler),
        brqr=_rot_companion_cols(f(b_rq) * scaler),
        wrk=np.concatenate([f(W_rk), _rot_companion_cols(f(W_rk))], axis=1),
        brk=np.concatenate([f(b_rk), _rot_companion_cols(f(b_rk))], axis=0),
        wuk=f(W_uk), buk=f(b_uk),
        wuv=f(W_uv), buv=f(b_uv),
        wo=f(W_o), bo=f(b_o),
        cosk=f(cosk), sink=f(sink),
        ones_in=np.ones((128, 128), np.float32),
    )
    shared = {k: np.ascontiguousarray(v) for k, v in shared.items()}
    mm_keys = {"wdq", "wdkv", "wuq", "wrq", "wrqr", "wrk", "wuk", "wuv", "wo",
               "buv", "bo", "ones_in"}
    if getattr(c, "bf16", False):
        import ml_dtypes
        for k in mm_keys:
            shared[k] = shared[k].astype(ml_dtypes.bfloat16)

    n_cores = 2 * B
    in_maps = []
    for core in range(n_cores):
        b, half = core // 2, core % 2
        xtc = np.ascontiguousarray(sequence[b].T)         # [E, S]
        q0 = half * c.Q
        xtqc = np.ascontiguousarray(xtc[:, q0:q0 + c.Q])  # [E, Q]
        cq = np.tile(np.repeat(cos[q0:q0 + c.Q].T, 2, axis=0), (2, 1))  # [128, Q]
        sq = np.tile(np.repeat(sin[q0:q0 + c.Q].T, 2, axis=0), (2, 1))
        m = dict(shared)
        if getattr(c, "bf16", False):
            import ml_dtypes
            xtc = xtc.astype(ml_dtypes.bfloat16)
            xtqc = xtqc.astype(ml_dtypes.bfloat16)
        m.update(xt=xtc, xtq=xtqc,
                 cosq=np.ascontiguousarray(cq), sinq=np.ascontiguousarray(sq))
        in_maps.append(m)
    return in_maps


_PROG_CACHE = {}


def kernel(**inputs) -> np.ndarray:
    from concourse.bass_utils import run_bass_kernel_spmd

    _install_wait_split_hook()

    cfg = FULL
    has_buv = bool(np.any(np.asarray(inputs["b_uv"])))
    has_bo = bool(np.any(np.asarray(inputs["b_o"])))
    key = ("full", has_buv, has_bo)
    if key not in _PROG_CACHE:
        _PROG_CACHE[key] = build_program(cfg, has_buv=has_buv, has_bo=has_bo)
    nc = _PROG_CACHE[key]

    in_maps = host_inputs(cfg, **inputs)
    n = len(in_maps)
    res = run_bass_kernel_spmd(nc, in_maps, list(range(n)))

    B = n // 2
    S = 2 * cfg.Q
    full = np.empty((B, S, cfg.DM), dtype=np.float32)
    for core in range(n):
        b, half = core // 2, core % 2
        full[b, half * cfg.Q:(half + 1) * cfg.Q, :] = res.results[core]["out"]
    return full



# revision 6
# speedup vs baseline: 1.0967x; 1.0967x over previous
"""Multi-Head Latent Attention (DeepSeek-style MLA) on 8 TRN2 NeuronCores.

Sharding: core c handles batch b = c//2 and query rows [ (c%2)*S/2, (c%2+1)*S/2 ).
Each core computes the full KV-side projections for its batch (duplicated between
the two cores sharing a batch) and the Q-side projections / attention / output
projection for its query half. No collectives; the host gathers the 8 output
shards.

Layout strategy: activations are kept feature-major ("transposed", [feature, seq])
so every matmul's contraction dim lands on SBUF partitions. Attention output is
produced directly as attT[h*128+d, q] (v as stationary operand, expT as moving),
which is exactly the lhsT layout the output projection needs - no PE transposes
anywhere.

Softmax denominator: exp tiles are accumulated over key-chunks on the (otherwise
idle) GpSimd engine; a single ones-matmul per (head, q-tile) does the partition
sum. The z/reciprocal/normalize tail is software-pipelined one (head, q-tile)
job behind the score/AV loop so the PE never waits on it. (The naive
per-key-chunk ones-matmul variant costs ~180us of PE time at this size.)

qT / roped q_rotT / attT stay resident in SBUF (bf16) instead of bouncing
through DRAM between phases; the output projection reads attT slices directly.

RoPE is folded into companion weight matrices host-side:
  rope(x)[2i]   = x[2i] cos_i - x[2i+1] sin_i
  rope(x)[2i+1] = x[2i+1] cos_i + x[2i] sin_i
so with xr = x @ Wr where Wr[:,2i] = -W[:,2i+1], Wr[:,2i+1] = W[:,2i]:
  rope(x @ W) = (x @ W) * cosP + (x @ Wr) * sinP   (pure elementwise).

Matmul dtypes are chosen empirically (measured on this hardware): float32r for
the projection/AV path (fastest), bf16 for the score operands and the output
projection (kT/qT/attT/wo), which also halves their SBUF/DMA footprint.
"""

import sys
import numpy as np

sys.path.insert(0, "/opt/trn_rl_repo")

from contextlib import ExitStack  # noqa: E402

import concourse.bass as bass  # noqa: E402
import concourse.mybir as mybir  # noqa: E402
import concourse.tile as tile  # noqa: E402

F32 = mybir.dt.float32
FR = mybir.dt.float32r
BF = mybir.dt.bfloat16
AF = mybir.ActivationFunctionType
ALU = mybir.AluOpType

# Max sync-waits walrus CoreV3 codegen accepts on one instruction. The stock
# TileContext tail-drain attaches one wait per outstanding semaphore to a
# single Drain, which this walrus build rejects ("Too many sync wait
# commands"); split across several drains instead.
_MAX_WAITS_PER_INST = 1


def _split_excess_waits_json(bir_json):
    """Walrus CoreV3 codegen rejects instructions carrying more than one
    sync-wait. Tile freely attaches several. Rewrite the BIR: keep one wait on
    the instruction, move the rest onto NoOps inserted just before it on the
    same engine (a same-engine wait that fires earlier is strictly safe).
    Updates are left untouched - they must fire at instruction completion."""
    import orjson

    bir = orjson.loads(bir_json)
    n = 0
    for fn in bir.get("functions", []):
        for bb in fn.get("blocks", []):
            out = []
            for inst in bb.get("instructions", []):
                si = inst.get("sync_info")
                waits = (si or {}).get("on_wait") or []
                if len(waits) > _MAX_WAITS_PER_INST:
                    keep = waits[-_MAX_WAITS_PER_INST:]
                    for w in waits[:-_MAX_WAITS_PER_INST]:
                        out.append({
                            "name": f"I-WS{n}",
                            "opcode": "NoOp",
                            "engine": inst["engine"],
                            "ins": [],
                            "outs": [],
                            "sync_info": {"on_update": [], "on_wait": [w]},
                        })
                        n += 1
                    si["on_wait"] = keep
                out.append(inst)
            bb["instructions"] = out
    return orjson.dumps(bir)


_COMPILE_HOOKED = False


def _install_wait_split_hook():
    """Wrap compile_bir_kernel (both the bass_utils global and the name
    bass2jax imported) so every BIR headed to walrus gets the wait split."""
    global _COMPILE_HOOKED
    if _COMPILE_HOOKED:
        return
    from concourse import bass2jax, bass_utils

    orig = bass_utils.compile_bir_kernel

    def hooked(bir_json, tmpdir, neff_name="file.neff"):
        return orig(_split_excess_waits_json(bir_json), tmpdir, neff_name=neff_name)

    bass_utils.compile_bir_kernel = hooked
    bass2jax.compile_bir_kernel = hooked
    _COMPILE_HOOKED = True


class SplitDrainTileContext(tile.TileContext):
    def _drain_and_barrier(self, tick_clock, wait_clock):
        from concourse.tile_scheduler import N_PROCS
        from concourse.vector_clock import ScopedClock, VectorClock

        g = tick_clock.global_clock
        vals = [g[p] for p in range(N_PROCS)]
        nz = [p for p in range(N_PROCS) if vals[p] > 0]
        groups = [nz[i:i + _MAX_WAITS_PER_INST]
                  for i in range(0, len(nz), _MAX_WAITS_PER_INST)] or [[]]
        for grp in groups:
            sub = VectorClock([vals[p] if p in grp else 0 for p in range(N_PROCS)])
            drain_inst = self.nc.sync.drain()
            wait_clock.add_sem_waits(drain_inst.ins, ScopedClock({None: sub}))

        self.nc.all_engine_barrier()
        assert self.sems is not None
        popped = self.nc._tile_sem_poison_stack.pop()
        assert popped is self._sem_poison
        self.nc.clear_and_free_semaphores(list(self.sems.allocated().values()))
        self.nc.all_engine_barrier()


# ----------------------------------------------------------------------------
# Config
# ----------------------------------------------------------------------------

class Cfg:
    def __init__(self, E=2048, DM=2048, H=16, DC=512, DC1=1536, S=2048, Q=1024,
                 QT=512):
        self.E, self.DM, self.H, self.DC, self.DC1 = E, DM, H, DC, DC1
        self.S, self.Q, self.QT = S, Q, QT
        self.DR = 64          # rotary dim (fixed by the problem)
        self.DH = 128         # nope head dim (fixed: DM // H)
        assert DM == H * self.DH and H % 2 == 0
        assert E % 128 == 0 and DC % 128 == 0 and DC1 % 128 == 0
        assert S % 128 == 0
        assert Q % QT == 0 and Q % 128 == 0 and QT <= 512
        self.EC = E // 128        # embed chunks
        self.CC = DC // 128       # c_kv chunks
        self.C1C = DC1 // 128     # c_q chunks
        self.KC = S // 128        # key chunks (128-wide)
        self.ST = min(512, S)     # seq tile for phase 1
        self.STN = S // self.ST
        self.NT = min(512, S)     # kT free tile
        self.NTN = S // self.NT
        self.QTN = Q // QT
        self.MT = min(512, DM)    # out-proj free tile
        self.MTN = DM // self.MT
        self.QON = Q // 128       # out-proj q tiles


FULL = Cfg()


# ----------------------------------------------------------------------------
# Program builder (single-core SPMD program)
# ----------------------------------------------------------------------------

def build_program(cfg: Cfg, has_buv=True, has_bo=True):
    c = cfg
    nc = bass.Bass()
    r = lambda ap: ap  # noqa: E731

    # -- DRAM parameters -----------------------------------------------------
    xt = nc.dram_tensor("xt", [c.E, c.S], FR, kind="ExternalInput")
    xtq = nc.dram_tensor("xtq", [c.E, c.Q], FR, kind="ExternalInput")
    cosq = nc.dram_tensor("cosq", [128, c.Q], F32, kind="ExternalInput")
    sinq = nc.dram_tensor("sinq", [128, c.Q], F32, kind="ExternalInput")
    cosk = nc.dram_tensor("cosk", [64, c.S], F32, kind="ExternalInput")
    sink = nc.dram_tensor("sink", [64, c.S], F32, kind="ExternalInput")
    wdq = nc.dram_tensor("wdq", [c.E, c.DC1], FR, kind="ExternalInput")
    bdq = nc.dram_tensor("bdq", [c.DC1], F32, kind="ExternalInput")
    wdkv = nc.dram_tensor("wdkv", [c.E, c.DC], FR, kind="ExternalInput")
    bdkv = nc.dram_tensor("bdkv", [c.DC], F32, kind="ExternalInput")
    wuq = nc.dram_tensor("wuq", [c.DC1, c.DM], FR, kind="ExternalInput")
    buq = nc.dram_tensor("buq", [c.DM], F32, kind="ExternalInput")
    wrq = nc.dram_tensor("wrq", [c.DC1, c.H * c.DR], FR, kind="ExternalInput")
    brq = nc.dram_tensor("brq", [c.H * c.DR], F32, kind="ExternalInput")
    wrqr = nc.dram_tensor("wrqr", [c.DC1, c.H * c.DR], FR, kind="ExternalInput")
    brqr = nc.dram_tensor("brqr", [c.H * c.DR], F32, kind="ExternalInput")
    wrk = nc.dram_tensor("wrk", [c.E, 2 * c.DR], FR, kind="ExternalInput")
    brk = nc.dram_tensor("brk", [2 * c.DR], F32, kind="ExternalInput")
    wuk = nc.dram_tensor("wuk", [c.DC, c.DM], FR, kind="ExternalInput")
    buk = nc.dram_tensor("buk", [c.DM], F32, kind="ExternalInput")
    wuv = nc.dram_tensor("wuv", [c.DC, c.DM], FR, kind="ExternalInput")
    buv = nc.dram_tensor("buv", [c.DM], FR, kind="ExternalInput")
    wo = nc.dram_tensor("wo", [c.DM, c.DM], BF, kind="ExternalInput")
    bo = nc.dram_tensor("bo", [c.DM], FR, kind="ExternalInput")
    ones_d = nc.dram_tensor("ones_in", [128, 128], FR, kind="ExternalInput")
    out = nc.dram_tensor("out", [c.Q, c.DM], F32, kind="ExternalOutput")

    with SplitDrainTileContext(nc) as tc, ExitStack() as ctx:
        # -- persistent pools ------------------------------------------------
        consts = ctx.enter_context(tc.tile_pool(name="consts", bufs=1))
        res = ctx.enter_context(tc.tile_pool(name="res", bufs=1))

        ckvT = res.tile([128, c.CC, c.S], FR, tag="ckvT")     # c_kv^T
        krT = res.tile([128, c.S], BF, tag="krT")             # roped k_rot^T, dup rows

        ones128 = consts.tile([128, 128], FR, tag="ones128")
        nc.sync.dma_start(out=ones128, in_=ones_d[:, :])
        ones1 = ones128[0:1, :]

        def load_pcol(name, vec, n):
            # [n*128] dram vector -> [128, n] sbuf (per-partition scalars)
            t = consts.tile([128, n], F32, tag=name)
            nc.sync.dma_start(out=t, in_=vec.rearrange("(c p) -> p c", p=128))
            return t

        bdq_sb = load_pcol("bdq", bdq, c.C1C)
        bdkv_sb = load_pcol("bdkv", bdkv, c.CC)
        buq_sb = load_pcol("buq", buq, c.H)
        brq_sb = load_pcol("brq", brq, c.H // 2)
        brqr_sb = load_pcol("brqr", brqr, c.H // 2)
        # packed k-rope bias: col 0 = brk[0:64], col 1 = companion brk[64:128],
        # both based at partition 0 (DVE ops need same start partition)
        brk_sb = consts.tile([64, 2], F32, tag="brk")
        nc.sync.dma_start(out=brk_sb, in_=brk.rearrange("(c p) -> p c", p=64))
        buk_sb = load_pcol("buk", buk, c.H)
        buv_sb = bo_sb = None
        if has_buv:
            buv_sb = consts.tile([1, c.DM], FR, tag="buv")
            nc.sync.dma_start(out=buv_sb, in_=buv[:].unsqueeze(0))
        if has_bo:
            bo_sb = consts.tile([1, c.DM], FR, tag="bo")
            nc.sync.dma_start(out=bo_sb, in_=bo[:].unsqueeze(0))

        # PSUM pools (8 banks total: 2+3+2+1)
        psA = ctx.enter_context(tc.tile_pool(name="psA", bufs=2, space="PSUM"))
        psS = ctx.enter_context(tc.tile_pool(name="psS", bufs=3, space="PSUM"))
        psG = ctx.enter_context(tc.tile_pool(name="psG", bufs=2, space="PSUM"))
        psZ = ctx.enter_context(tc.tile_pool(name="psZ", bufs=1, space="PSUM"))

        # ==================================================================
        # Phase 1a: c_kv^T and roped k_rot^T over the full sequence
        # ==================================================================
        with tc.tile_pool(name="p1ax", bufs=c.EC + 4) as p1ax, \
             tc.tile_pool(name="p1aw", bufs=c.EC) as p1aw, \
             tc.tile_pool(name="p1am", bufs=1) as p1am, \
             tc.tile_pool(name="p1at", bufs=4) as p1at:

            cosk_sb = p1am.tile([64, c.S], F32, tag="cosk")
            sink_sb = p1am.tile([64, c.S], F32, tag="sink")
            nc.sync.dma_start(out=cosk_sb, in_=cosk[:, :])
            nc.sync.dma_start(out=sink_sb, in_=sink[:, :])

            wdkv_t, wrk_t = [], []
            for e in range(c.EC):
                wt = p1aw.tile([128, c.DC], FR, tag="wdkv")
                nc.sync.dma_start(out=wt, in_=wdkv[e * 128:(e + 1) * 128, :])
                wdkv_t.append(wt)
                rt = p1aw.tile([128, 2 * c.DR], FR, tag="wrk")
                nc.sync.dma_start(out=rt, in_=wrk[e * 128:(e + 1) * 128, :])
                wrk_t.append(rt)

            for st in range(c.STN):
                ssl = bass.ts(st, c.ST)
                xts = []
                for e in range(c.EC):
                    t = p1ax.tile([128, c.ST], FR, tag="xt")
                    nc.sync.dma_start(out=t, in_=xt[e * 128:(e + 1) * 128, ssl])
                    xts.append(t)
                for ct in range(c.CC):
                    ps = psA.tile([128, c.ST], F32, tag="ps")
                    for e in range(c.EC):
                        nc.tensor.matmul(ps, r(wdkv_t[e][:, ct * 128:(ct + 1) * 128]),
                                         r(xts[e]), start=(e == 0), stop=(e == c.EC - 1))
                    nc.vector.tensor_scalar_add(ckvT[:, ct, ssl], ps,
                                                bdkv_sb[:, ct:ct + 1])
                # k_rot: A rows and Ar rows in separate psums (partition-aligned)
                psa = psA.tile([64, c.ST], F32, tag="ps")
                for e in range(c.EC):
                    nc.tensor.matmul(psa, r(wrk_t[e][:, 0:c.DR]), r(xts[e]),
                                     start=(e == 0), stop=(e == c.EC - 1))
                psar = psA.tile([64, c.ST], F32, tag="ps")
                for e in range(c.EC):
                    nc.tensor.matmul(psar, r(wrk_t[e][:, c.DR:2 * c.DR]), r(xts[e]),
                                     start=(e == 0), stop=(e == c.EC - 1))
                tmp = p1at.tile([64, c.ST], F32, tag="ktmp")
                nc.vector.scalar_tensor_tensor(tmp, psa, brk_sb[:, 0:1],
                                               cosk_sb[:, ssl], ALU.add, ALU.mult)
                nc.vector.scalar_tensor_tensor(krT[0:64, ssl], psar,
                                               brk_sb[:, 1:2],
                                               sink_sb[:, ssl], ALU.add, ALU.mult)
                nc.vector.tensor_add(krT[0:64, ssl], krT[0:64, ssl], tmp)
            # duplicate kr rows so odd heads can matmul at base_partition 64
            nc.sync.dma_start(out=krT[64:128, :], in_=krT[0:64, :])

        # ==================================================================
        # Phase 1b/1c: c_q^T, then q^T (scaled) and roped q_rot^T -> SBUF
        # ==================================================================
        with tc.tile_pool(name="pcq", bufs=1) as pcq:
            cqT = pcq.tile([128, c.C1C, c.Q], FR, tag="cqT")

            with tc.tile_pool(name="p1bx", bufs=c.QTN * c.EC + 2) as p1bx, \
                 tc.tile_pool(name="p1bw", bufs=3) as p1bw:
                # all query-tile activations resident so wdq streams ONCE
                xqs = {}
                for qt in range(c.QTN):
                    qsl = bass.ts(qt, c.QT)
                    for e in range(c.EC):
                        t = p1bx.tile([128, c.QT], FR, tag="xq")
                        nc.sync.dma_start(out=t, in_=xtq[e * 128:(e + 1) * 128, qsl])
                        xqs[qt, e] = t
                for ct in range(c.C1C):
                    wdq_ct = p1bw.tile([128, c.EC, 128], FR, tag="wdq")
                    nc.sync.dma_start(
                        out=wdq_ct,
                        in_=wdq.rearrange("(e p) m -> p e m", p=128)[:, :, ct * 128:(ct + 1) * 128])
                    for qt in range(c.QTN):
                        qsl = bass.ts(qt, c.QT)
                        ps = psA.tile([128, c.QT], F32, tag="ps")
                        for e in range(c.EC):
                            nc.tensor.matmul(ps, r(wdq_ct[:, e, :]), r(xqs[qt, e]),
                                             start=(e == 0), stop=(e == c.EC - 1))
                        nc.vector.tensor_scalar_add(cqT[:, ct, qsl], ps,
                                                    bdq_sb[:, ct:ct + 1])

            # persistent q-side results: right-side SBUF stack so the left
            # stack's LIFO order (pcq releasing before this) is preserved
            qres = ctx.enter_context(tc.tile_pool(name="qres", bufs=1, side="right"))
            qT = qres.tile([128, c.H, c.Q], BF, tag="qT")
            qrT = qres.tile([128, c.H // 2, c.Q], BF, tag="qrT")

            with tc.tile_pool(name="p1cw", bufs=2) as p1cw, \
                 tc.tile_pool(name="p1cm", bufs=1) as p1cm, \
                 tc.tile_pool(name="p1ct", bufs=4) as p1ct:

                cosq_sb = p1cm.tile([128, c.Q], F32, tag="cosq")
                sinq_sb = p1cm.tile([128, c.Q], F32, tag="sinq")
                nc.sync.dma_start(out=cosq_sb, in_=cosq[:, :])
                nc.sync.dma_start(out=sinq_sb, in_=sinq[:, :])

                for h in range(c.H):
                    wuq_h = p1cw.tile([128, c.C1C, 128], FR, tag="wuq")
                    nc.sync.dma_start(
                        out=wuq_h,
                        in_=wuq.rearrange("(cc p) m -> p cc m", p=128)[:, :, h * 128:(h + 1) * 128])
                    for qt in range(c.QTN):
                        qsl = bass.ts(qt, c.QT)
                        ps = psA.tile([128, c.QT], F32, tag="ps")
                        for ct in range(c.C1C):
                            nc.tensor.matmul(ps, r(wuq_h[:, ct, :]), r(cqT[:, ct, qsl]),
                                             start=(ct == 0), stop=(ct == c.C1C - 1))
                        nc.vector.tensor_scalar_add(qT[:, h, qsl], ps,
                                                    buq_sb[:, h:h + 1])
                for hp in range(c.H // 2):
                    wrq_hp = p1cw.tile([128, c.C1C, 128], FR, tag="wrq")
                    nc.sync.dma_start(
                        out=wrq_hp,
                        in_=wrq.rearrange("(cc p) m -> p cc m", p=128)[:, :, hp * 128:(hp + 1) * 128])
                    wrqr_hp = p1cw.tile([128, c.C1C, 128], FR, tag="wrqr")
                    nc.sync.dma_start(
                        out=wrqr_hp,
                        in_=wrqr.rearrange("(cc p) m -> p cc m", p=128)[:, :, hp * 128:(hp + 1) * 128])
                    for qt in range(c.QTN):
                        qsl = bass.ts(qt, c.QT)
                        psa = psA.tile([128, c.QT], F32, tag="ps")
                        for ct in range(c.C1C):
                            nc.tensor.matmul(psa, r(wrq_hp[:, ct, :]), r(cqT[:, ct, qsl]),
                                             start=(ct == 0), stop=(ct == c.C1C - 1))
                        psar = psA.tile([128, c.QT], F32, tag="ps")
                        for ct in range(c.C1C):
                            nc.tensor.matmul(psar, r(wrqr_hp[:, ct, :]), r(cqT[:, ct, qsl]),
                                             start=(ct == 0), stop=(ct == c.C1C - 1))
                        tmp = p1ct.tile([128, c.QT], F32, tag="qtmp")
                        nc.vector.scalar_tensor_tensor(tmp, psa, brq_sb[:, hp:hp + 1],
                                                       cosq_sb[:, qsl], ALU.add, ALU.mult)
                        nc.vector.scalar_tensor_tensor(qrT[:, hp, qsl], psar,
                                                       brqr_sb[:, hp:hp + 1],
                                                       sinq_sb[:, qsl], ALU.add, ALU.mult)
                        nc.vector.tensor_add(qrT[:, hp, qsl], qrT[:, hp, qsl], tmp)

        # attention output, resident in SBUF (reuses the freed cqT space)
        attp = ctx.enter_context(tc.tile_pool(name="attp", bufs=1))
        attT = attp.tile([128, c.H, c.Q], BF, tag="attT")

        # out-proj pools open early: mt=0's wo tiles prefetch during the
        # attention phase (DMA queues are ~90% idle there), so phase 3
        # starts without a weight-load stall.
        ow = ctx.enter_context(tc.tile_pool(name="ow", bufs=c.H + 6))
        oo = ctx.enter_context(tc.tile_pool(name="oo", bufs=3))
        wo_pre = []
        for hc in range(c.H):
            t = ow.tile([128, c.MT], BF, tag="wo")
            nc.sync.dma_start(out=t, in_=wo[hc * 128:(hc + 1) * 128, 0:c.MT])
            wo_pre.append(t)

        # ==================================================================
        # Phase 2: per-head attention
        # ==================================================================
        with tc.tile_pool(name="hw", bufs=2) as hw, \
             tc.tile_pool(name="hk", bufs=2) as hk, \
             tc.tile_pool(name="hv", bufs=3) as hv, \
             tc.tile_pool(name="he", bufs=4) as he, \
             tc.tile_pool(name="hsum", bufs=2) as hsum, \
             tc.tile_pool(name="hr", bufs=2) as hr:

            # pending tail of the previous (head, q-tile) job:
            # (head, qsl, gps, etsum)
            pending = [None]

            def flush_tail():
                if pending[0] is None:
                    return
                ph, pqsl, pgps, petsum = pending[0]
                pending[0] = None
                zps = psZ.tile([128, c.QT], F32, tag="z")
                nc.tensor.matmul(zps, r(ones128), r(petsum), start=True, stop=True)
                recip = hr.tile([128, c.QT], F32, tag="recip")
                nc.vector.reciprocal(recip, zps)
                nc.vector.tensor_mul(attT[:, ph, pqsl], pgps, recip)

            v_tiles = {}
            for h in range(c.H):
                hp, par = h // 2, (h % 2) * 64
                if h % 2 == 0:
                    # v for the head pair: [s, 2*128], 128 s-rows at a time
                    wuv_hp = hw.tile([128, c.CC, 256], FR, tag="wuv")
                    nc.sync.dma_start(
                        out=wuv_hp,
                        in_=wuv.rearrange("(cc p) m -> p cc m", p=128)[:, :, hp * 256:(hp + 1) * 256])
                    v0 = hv.tile([128, c.KC, 128], FR, tag="vh")
                    v1 = hv.tile([128, c.KC, 128], FR, tag="vh")
                    for st in range(c.KC):
                        ps = psA.tile([128, 256], F32, tag="ps")
                        for cc in range(c.CC):
                            nc.tensor.matmul(ps, r(ckvT[:, cc, st * 128:(st + 1) * 128]),
                                             r(wuv_hp[:, cc, :]),
                                             start=(cc == 0),
                                             stop=(not has_buv and cc == c.CC - 1))
                        if has_buv:
                            nc.tensor.matmul(ps, r(ones1),
                                             r(buv_sb[:, hp * 256:(hp + 1) * 256]),
                                             start=False, stop=True)
                        nc.vector.tensor_copy(v0[:, st, :], ps[:, 0:128])
                        nc.vector.tensor_copy(v1[:, st, :], ps[:, 128:256])
                    v_tiles[h], v_tiles[h + 1] = v0, v1

                # kT for this head: [128 d, S] (bf16: score lhsT)
                wuk_h = hw.tile([128, c.CC, 128], FR, tag="wuk")
                nc.sync.dma_start(
                    out=wuk_h,
                    in_=wuk.rearrange("(cc p) m -> p cc m", p=128)[:, :, h * 128:(h + 1) * 128])
                kT = hk.tile([128, c.S], BF, tag="kT")
                for nt in range(c.NTN):
                    nsl = bass.ts(nt, c.NT)
                    ps = psA.tile([128, c.NT], F32, tag="ps")
                    for cc in range(c.CC):
                        nc.tensor.matmul(ps, r(wuk_h[:, cc, :]), r(ckvT[:, cc, nsl]),
                                         start=(cc == 0), stop=(cc == c.CC - 1))
                    nc.vector.tensor_scalar_add(kT[:, nsl], ps, buk_sb[:, h:h + 1])

                vh = v_tiles[h]
                for qt in range(c.QTN):
                    qsl = bass.ts(qt, c.QT)
                    gps = psG.tile([128, c.QT], F32, tag="g")
                    etsum = hsum.tile([128, c.QT], FR, tag="etsum")
                    for kc in range(c.KC):
                        ksl = bass.ts(kc, 128)
                        sps = psS.tile([128, c.QT], F32, tag="s")
                        nc.tensor.matmul(sps, r(kT[:, ksl]), r(qT[:, h, qsl]),
                                         start=True, stop=False)
                        nc.tensor.matmul(sps, r(krT[par:par + 64, ksl]),
                                         r(qrT[par:par + 64, hp, qsl]),
                                         start=False, stop=True)
                        et = he.tile([128, c.QT], FR, tag="e")
                        nc.scalar.activation(et, sps, AF.Exp)
                        if kc == 0:
                            nc.gpsimd.tensor_copy(etsum, et)
                        else:
                            nc.gpsimd.tensor_add(etsum, etsum, et)
                        nc.tensor.matmul(gps, r(vh[:, kc, :]), r(et),
                                         start=(kc == 0), stop=(kc == c.KC - 1))
                        if kc == 2:
                            flush_tail()
                    pending[0] = (h, qsl, gps, etsum)
            flush_tail()

        # ==================================================================
        # Phase 3: output projection  out[q, m] = attT.T @ wo + bo
        # ==================================================================
        for mt in range(c.MTN):
            msl = bass.ts(mt, c.MT)
            if mt == 0:
                wo_t = wo_pre
            else:
                wo_t = []
                for hc in range(c.H):
                    t = ow.tile([128, c.MT], BF, tag="wo")
                    nc.sync.dma_start(out=t, in_=wo[hc * 128:(hc + 1) * 128, msl])
                    wo_t.append(t)
            for qo in range(c.QON):
                ps = psA.tile([128, c.MT], F32, tag="ps")
                for hc in range(c.H):
                    nc.tensor.matmul(ps, r(attT[:, hc, qo * 128:(qo + 1) * 128]),
                                     r(wo_t[hc]),
                                     start=(hc == 0),
                                     stop=(not has_bo and hc == c.H - 1))
                if has_bo:
                    nc.tensor.matmul(ps, r(ones1), r(bo_sb[:, msl]),
                                     start=False, stop=True)
                osb = oo.tile([128, c.MT], F32, tag="osb")
                nc.vector.tensor_copy(osb, ps)
                nc.sync.dma_start(out=out[qo * 128:(qo + 1) * 128, msl], in_=osb)

    return nc


# ----------------------------------------------------------------------------
# Host side: input prep, sharding, gather
# ----------------------------------------------------------------------------

def _rope_tables(seq_len, dim, theta=10000.0):
    inv_freq = 1.0 / (theta ** (np.arange(0, dim, 2, dtype=np.float32) / dim))
    t = np.arange(seq_len, dtype=np.float32)
    ang = t[:, None] * inv_freq[None, :]  # [S, dim/2]
    return np.cos(ang).astype(np.float32), np.sin(ang).astype(np.float32)


def _rot_companion_cols(w):
    """wr[..., 2i] = -w[..., 2i+1]; wr[..., 2i+1] = w[..., 2i]."""
    wr = np.empty_like(w)
    wr[..., 0::2] = -w[..., 1::2]
    wr[..., 1::2] = w[..., 0::2]
    return wr


def host_inputs(cfg, sequence, W_dkv, b_dkv, W_dq, b_dq, W_uq, b_uq, W_uk, b_uk,
                W_uv, b_uv, W_rq, b_rq, W_rk, b_rk, W_o, b_o):
    """Build the per-core input maps for the SPMD program."""
    import ml_dtypes
    c = cfg
    f = lambda a: np.ascontiguousarray(np.asarray(a, dtype=np.float32))  # noqa: E731
    sequence = f(sequence)
    B = sequence.shape[0]
    scaler = np.float32(1.0 / np.sqrt(c.DH + c.DR))

    cos, sin = _rope_tables(c.S, c.DR)  # [S, 32]
    # rows 2i and 2i+1 both carry table column i
    cosk = np.repeat(cos.T, 2, axis=0)  # [64, S]
    sink = np.repeat(sin.T, 2, axis=0)

    shared = dict(
        wdq=f(W_dq), bdq=f(b_dq),
        wdkv=f(W_dkv), bdkv=f(b_dkv),
        wuq=f(W_uq) * scaler, buq=f(b_uq) * scaler,
        wrq=f(W_rq) * scaler, brq=f(b_rq) * scaler,
        wrqr=_rot_companion_cols(f(W_rq) * scaler),
        brqr=_rot_companion_cols(f(b_rq) * scaler),
        wrk=np.concatenate([f(W_rk), _rot_companion_cols(f(W_rk))], axis=1),
        brk=np.concatenate([f(b_rk), _rot_companion_cols(f(b_rk))], axis=0),
        wuk=f(W_uk), buk=f(b_uk),
        wuv=f(W_uv), buv=f(b_uv),
        wo=f(W_o).astype(ml_dtypes.bfloat16), bo=f(b_o),
        cosk=f(cosk), sink=f(sink),
        ones_in=np.ones((128, 128), np.float32),
    )
    shared = {k: np.ascontiguousarray(v) for k, v in shared.items()}

    n_cores = 2 * B
    in_maps = []
    for core in range(n_cores):
        b, half = core // 2, core % 2
        xtc = np.ascontiguousarray(sequence[b].T)         # [E, S]
        q0 = half * c.Q
        xtqc = np.ascontiguousarray(xtc[:, q0:q0 + c.Q])  # [E, Q]
        cq = np.tile(np.repeat(cos[q0:q0 + c.Q].T, 2, axis=0), (2, 1))  # [128, Q]
        sq = np.tile(np.repeat(sin[q0:q0 + c.Q].T, 2, axis=0), (2, 1))
        m = dict(shared)
        m.update(xt=xtc, xtq=xtqc,
                 cosq=np.ascontiguousarray(cq), sinq=np.ascontiguousarray(sq))
        in_maps.append(m)
    return in_maps


_PROG_CACHE = {}


def kernel(**inputs) -> np.ndarray:
    from concourse.bass_utils import run_bass_kernel_spmd

    _install_wait_split_hook()

    cfg = FULL
    has_buv = bool(np.any(np.asarray(inputs["b_uv"])))
    has_bo = bool(np.any(np.asarray(inputs["b_o"])))
    key = ("full", has_buv, has_bo)
    if key not in _PROG_CACHE:
        _PROG_CACHE[key] = build_program(cfg, has_buv=has_buv, has_bo=has_bo)
    nc = _PROG_CACHE[key]

    in_maps = host_inputs(cfg, **inputs)
    n = len(in_maps)
    res = run_bass_kernel_spmd(nc, in_maps, list(range(n)))

    B = n // 2
    S = 2 * cfg.Q
    full = np.empty((B, S, cfg.DM), dtype=np.float32)
    for core in range(n):
        b, half = core // 2, core % 2
        full[b, half * cfg.Q:(half + 1) * cfg.Q, :] = res.results[core]["out"]
    return full
